# revision 45
# baseline (speedup 1.0000x reference)
"""Trainium2 Bass kernel for 3-layer GraphSAGE-mean (DenseGCN variant).

Strategy (8 NeuronCores, graph/data parallel, v2):
  - Nodes sharded by range (6250/core); edges binned by (dst 128-block,
    src half) with uniform per-(block,half) chunk budgets t_lo/t_hi
    (int16 gather-index limit forces the src-half split).
  - L1 does NOT gather on device: the host bakes the edge-major stream
    [x_hi|x_lo][src] in exact slot layout; the kernel streams it with
    sequential HWDGE DMA (no Q7 descriptor generation).
  - The L1 stream rows are pre-scaled by 1/deg(dst) on the host (hi/lo
    bf16 split around the scaled value), so L1 uses a plain 0/1 onehot.
  - L2/L3 aggregate via gpsimd.dma_gather of 256B bf16 rows from the
    AllGathered tables T2=[h1|xp], T3=[h2]. Gather Q7 descriptor-gen
    time scales with the static num_idxs (~10ns/row) and is the kernel's
    critical path; pads use idx 0 and are killed by dstloc=-1.
  - Scatter = onehot matmuls accumulating [feat, dst] sums in PSUM.
    Onehots are generated BATCHED (one is_equal tensor_tensor per block
    over [128, t_tot, 128] with broadcast dstloc) and for L2/L3 scaled
    by 1/deg(dst) via a second batched mult, so the PSUM result is the
    mean directly; Wl/Wr/bias paths then accumulate into a single PSUM
    tile and the epilogue is just a ReLU.
  - Means of h1/xp computed at L2 (s12T) are reused at L3. Transposes
    to feature-major (xpT/h1T) are hoisted out of the L1 block loop to
    keep PE runs long (p-state ramp).
"""

import sys

sys.path.insert(0, "/opt/trn_rl_repo")

import os

import numpy as np
import ml_dtypes

import concourse.bass as bass
import concourse.bacc as bacc
import concourse.tile as tile
from concourse import mybir
from concourse.bass import broadcast_tensor_aps
from concourse.bass_utils import run_bass_kernel_spmd

BF16 = ml_dtypes.bfloat16

N = 50000
E = 800000
NCORES = 8
SHARD = N // NCORES          # 6250
P = 128
NBLK = (SHARD + P - 1) // P  # 49
SHARD_PAD = NBLK * P         # 6272
HALF = N // 2                # 25000

F0 = 64
O1, O2, O3 = 64, 128, 128
FT = 128                     # table feature width (bf16 -> 256B rows)
LAYERS = int(os.environ.get("KERNEL_LAYERS", "3"))

# Max chunks (x128 rows) per dma_gather call. 128 rows = 8 descriptors per
# SDMA engine. Must be small enough that every call keeps at least one
# valid (non-pad) slot on every core (per-call exact counts are baked on
# the host), and large enough to amortize the ~0.7us fixed Q7 cost.
GSPLIT = int(os.environ.get("KERNEL_GSPLIT", "5"))


def _call_ranges(t_lo: int, t_hi: int):
    """Chunk ranges [(c0, c1), ...] for one block's gather calls (lo then hi)."""
    t_tot = t_lo + t_hi
    out = []
    for c0, c1 in ((0, t_lo), (t_lo, t_tot)):
        step = (c1 - c0) if not GSPLIT else GSPLIT
        for s0 in range(c0, c1, step):
            out.append((s0, min(s0 + step, c1)))
    return out

dt = mybir.dt


def _bf16(x):
    return np.ascontiguousarray(np.asarray(x).astype(BF16))


def _wrap16(idx_flat: np.ndarray) -> np.ndarray:
    """[n] -> [128, n/16]: slot i at [i%16, i//16], replicated over 8 q7 cores."""
    n = idx_flat.shape[0]
    w = idx_flat.reshape(n // 16, 16).T
    return np.ascontiguousarray(np.tile(w, (8, 1)))


def _preprocess(edge_index: np.ndarray):
    """Bin edges -> per-core slot layout: gather idx, dstloc, slotinv, and
    the edge->slot permutation for baking the L1 stream."""
    src = edge_index[0].astype(np.int64)
    dst = edge_index[1].astype(np.int64)
    cnt = np.bincount(dst, minlength=N)
    inv = (1.0 / np.maximum(cnt, 1)).astype(np.float32)

    core = dst // SHARD
    blk = (dst % SHARD) // P
    half = (src >= HALF).astype(np.int64)
    lsrc = src - half * HALF
    key = (core * NBLK + blk) * 2 + half
    counts = np.bincount(key, minlength=NCORES * NBLK * 2)
    t_lo = int(np.ceil(counts[0::2].max() / P))
    t_hi = int(np.ceil(counts[1::2].max() / P))
    t_tot = t_lo + t_hi
    TS = NBLK * t_tot * P    # slots per core
    # per-(block, half) slot budget = max count over cores (NOT rounded to
    # 128): gather gen time scales with the static num_idxs, so trimming
    # the budget trims Q7 time directly. Baked into the program (SPMD).
    nbh = counts.reshape(NCORES, NBLK, 2).max(axis=0).astype(np.int64)

    order = np.argsort(key, kind="stable")
    skey = key[order]
    ssrc = src[order]
    slsrc = lsrc[order]
    sdst = dst[order]
    group_start = np.searchsorted(skey, np.arange(NCORES * NBLK * 2))
    pos = np.arange(E) - group_start[skey]

    scoreb = skey // 2
    shalf = skey % 2
    slot = (scoreb % NBLK) * (t_tot * P) + shalf * (t_lo * P) + pos
    score = scoreb // NBLK

    cores = []
    for c in range(NCORES):
        m = score == c
        sl = slot[m]
        # pad idx = 0 (valid row; contribution killed by dl=-1). Gather gen
        # time scales with the static num_idxs, so pads aren't worth skipping.
        sidx = np.zeros(TS, np.int16)
        local_src = slsrc[m]
        assert local_src.max(initial=0) < 32768
        sidx[sl] = local_src.astype(np.int16)
        sdl = np.full(TS, -1.0, np.float32)
        sdl[sl] = (sdst[m] % SHARD % P).astype(np.float32)
        sinv = np.zeros(TS, np.float32)
        sinv[sl] = inv[sdst[m]]

        # wrap idx per (block, half) region
        sidx3 = sidx.reshape(NBLK, t_tot * P)
        wr = np.empty((128, NBLK * t_tot * 8), np.int16)
        for b in range(NBLK):
            lo = _wrap16(sidx3[b, : t_lo * P])
            hi = _wrap16(sidx3[b, t_lo * P :])
            wr[:, b * t_tot * 8 : b * t_tot * 8 + t_lo * 8] = lo
            wr[:, b * t_tot * 8 + t_lo * 8 : (b + 1) * t_tot * 8] = hi

        # dl/iv [128, NBLK*t_tot, 1]: [p, b*t_tot+cc, 0] = val of slot cc*128+p
        dl = np.ascontiguousarray(
            sdl.reshape(NBLK * t_tot, P).T[:, :, None].astype(BF16)
        )
        iv = np.ascontiguousarray(
            sinv.reshape(NBLK * t_tot, P).T[:, :, None].astype(BF16)
        )
        # edge -> slot map for stream baking
        cores.append(
            dict(
                idx=wr,
                dl=dl,
                iv=iv,
                slots=sl,
                srcs=ssrc[m],
                sinv=inv[sdst[m]],
            )
        )

    return cores, t_lo, t_hi, nbh


def _build_nc(t_lo: int, t_hi: int, nbh: np.ndarray):
    t_tot = t_lo + t_hi
    nc = bacc.Bacc("TRN2", target_bir_lowering=False, debug=False)

    # ---- I/O ----
    stream1_in = nc.dram_tensor(
        "stream1", [128, NBLK * t_tot, FT], dt.bfloat16, kind="ExternalInput"
    )
    xt_in = nc.dram_tensor("xt", [F0, SHARD_PAD], dt.bfloat16, kind="ExternalInput")
    idx_in = nc.dram_tensor(
        "idx", [128, NBLK * t_tot * 8], dt.int16, kind="ExternalInput"
    )
    dl_in = nc.dram_tensor("dl", [128, NBLK * t_tot, 1], dt.bfloat16, kind="ExternalInput")
    iv_in = nc.dram_tensor("iv", [128, NBLK * t_tot, 1], dt.bfloat16, kind="ExternalInput")
    ranges = _call_ranges(t_lo, t_hi)
    iota_in = nc.dram_tensor("iota3", [128, t_tot, 128], dt.bfloat16, kind="ExternalInput")
    ident_in = nc.dram_tensor("ident", [128, 128], dt.bfloat16, kind="ExternalInput")

    wp_in = nc.dram_tensor("Wp", [F0, O1], dt.bfloat16, kind="ExternalInput")
    bp_in = nc.dram_tensor("bp", [1, O1], dt.bfloat16, kind="ExternalInput")
    w1c_in = nc.dram_tensor("W1c", [128, O1], dt.bfloat16, kind="ExternalInput")
    bl1_in = nc.dram_tensor("bl1", [1, O1], dt.bfloat16, kind="ExternalInput")
    wr1_in = nc.dram_tensor("Wr1", [F0, O1], dt.bfloat16, kind="ExternalInput")
    w2c_in = nc.dram_tensor("W2c", [128, O2], dt.bfloat16, kind="ExternalInput")
    bl2_in = nc.dram_tensor("bl2", [1, O2], dt.bfloat16, kind="ExternalInput")
    wr2a_in = nc.dram_tensor("Wr2a", [64, O2], dt.bfloat16, kind="ExternalInput")
    wr2b_in = nc.dram_tensor("Wr2b", [64, O2], dt.bfloat16, kind="ExternalInput")
    w3c_in = nc.dram_tensor("W3c", [128, O3], dt.bfloat16, kind="ExternalInput")
    wl3h2_in = nc.dram_tensor("Wl3h2", [128, O3], dt.bfloat16, kind="ExternalInput")
    bl3_in = nc.dram_tensor("bl3", [1, O3], dt.bfloat16, kind="ExternalInput")
    wr3a_in = nc.dram_tensor("Wr3a", [64, O3], dt.bfloat16, kind="ExternalInput")
    wr3b_in = nc.dram_tensor("Wr3b", [64, O3], dt.bfloat16, kind="ExternalInput")
    wr3c_in = nc.dram_tensor("Wr3c", [128, O3], dt.bfloat16, kind="ExternalInput")

    h3_out = nc.dram_tensor("h3", [SHARD, O3], dt.float32, kind="ExternalOutput")

    AOP = mybir.AluOpType

    with tile.TileContext(nc) as tc:
        with (
            tc.tile_pool(name="cons", bufs=1) as cons,
            tc.tile_pool(name="st", bufs=2) as stp,
            tc.tile_pool(name="ohp", bufs=2) as ohp,
            tc.tile_pool(name="ohsp", bufs=3) as ohsp,
            tc.tile_pool(name="sb", bufs=2) as sb,
            tc.tile_pool(name="psum", bufs=2, space="PSUM") as ps,
            tc.tile_pool(name="dram", bufs=1, space="DRAM") as dr,
        ):
            # ---- constants -> SBUF ----
            iota_t = cons.tile([128, t_tot, 128], dt.bfloat16)
            nc.sync.dma_start(iota_t[:], iota_in[:])
            ident_t = cons.tile([128, 128], dt.bfloat16)
            nc.sync.dma_start(ident_t[:], ident_in[:])
            idx_t = cons.tile([128, NBLK * t_tot * 8], dt.int16)
            nc.sync.dma_start(idx_t[:], idx_in[:])
            dl_t = cons.tile([128, NBLK * t_tot, 1], dt.bfloat16)
            nc.sync.dma_start(dl_t[:], dl_in[:])
            iv_t = cons.tile([128, NBLK * t_tot, 1], dt.bfloat16)
            nc.sync.dma_start(iv_t[:], iv_in[:])
            xt_t = cons.tile([F0, SHARD_PAD], dt.bfloat16)
            nc.sync.dma_start(xt_t[:], xt_in[:])
            ones_t = cons.tile([1, 128], dt.bfloat16)
            nc.gpsimd.memset(ones_t[:], 1.0)

            def load_w(name, src, shape):
                t = cons.tile(list(shape), dt.bfloat16, name=name)
                nc.sync.dma_start(t[:], src[:])
                return t

            wp_t = load_w("wp_t", wp_in, (F0, O1))
            bp_t = load_w("bp_t", bp_in, (1, O1))
            w1c_t = load_w("w1c_t", w1c_in, (128, O1))
            bl1_t = load_w("bl1_t", bl1_in, (1, O1))
            wr1_t = load_w("wr1_t", wr1_in, (F0, O1))
            w2c_t = load_w("w2c_t", w2c_in, (128, O2))
            bl2_t = load_w("bl2_t", bl2_in, (1, O2))
            wr2a_t = load_w("wr2a_t", wr2a_in, (64, O2))
            wr2b_t = load_w("wr2b_t", wr2b_in, (64, O2))
            w3c_t = load_w("w3c_t", w3c_in, (128, O3))
            wl3h2_t = load_w("wl3h2_t", wl3h2_in, (128, O3))
            bl3_t = load_w("bl3_t", bl3_in, (1, O3))
            wr3a_t = load_w("wr3a_t", wr3a_in, (64, O3))
            wr3b_t = load_w("wr3b_t", wr3b_in, (64, O3))
            wr3c_t = load_w("wr3c_t", wr3c_in, (128, O3))

            # ---- persistent feature-major activations (local shard) ----
            xpT = cons.tile([O1, SHARD_PAD], dt.bfloat16)
            h1T = cons.tile([O1, SHARD_PAD], dt.bfloat16)
            h2T = cons.tile([O2, SHARD_PAD], dt.bfloat16)
            s12T = cons.tile([128, SHARD_PAD], dt.bfloat16)  # [mean_h1; mean_xp]
            t2sb = cons.tile([128, NBLK, 128], dt.bfloat16)  # staging [h1|xp]
            t3sb = cons.tile([128, NBLK, 128], dt.bfloat16)  # staging [h2]

            # gather double buffers (memset once: pad slots are never written)
            yga = cons.tile([128, t_tot, 128], dt.bfloat16)
            ygb = cons.tile([128, t_tot, 128], dt.bfloat16)
            nc.gpsimd.memset(yga[:], 0.0)
            nc.gpsimd.memset(ygb[:], 0.0)

            # ---- DRAM tables ----
            t2_shard = dr.tile([SHARD, FT], dt.bfloat16)
            t2_full = dr.tile([N, FT], dt.bfloat16, addr_space="Shared")
            t3_shard = dr.tile([SHARD, FT], dt.bfloat16)
            t3_full = dr.tile([N, FT], dt.bfloat16, addr_space="Shared")

            def gather_block(yg, b, table):
                # per-call num_idxs trimmed to the (block, half) max count
                # over cores -- gen time follows the static num_idxs.
                ibase = b * t_tot * 8
                for s0, s1 in ranges:
                    if s0 < t_lo:
                        tbl, n_half, h0 = table[0:HALF, :], int(nbh[b][0]), 0
                    else:
                        tbl, n_half, h0 = table[HALF:N, :], int(nbh[b][1]), t_lo
                    n_call = min((s1 - s0) * P, n_half - (s0 - h0) * P)
                    if n_call <= 0:
                        continue
                    s1e = s0 + (n_call + P - 1) // P
                    nc.gpsimd.dma_gather(
                        yg[:, s0:s1e, :],
                        tbl,
                        idx_t[:, ibase + s0 * 8 : ibase + s0 * 8 + (n_call + 15) // 16],
                        n_call,
                        n_call,
                        FT,
                    )

            def store_block(shard, staging, b):
                rows = min(P, SHARD - b * P)
                nc.sync.dma_start(
                    shard[b * P : b * P + rows, :], staging[:rows, b, :]
                )

            def allgather(shard, full):
                nc.gpsimd.collective_compute(
                    "AllGather",
                    mybir.AluOpType.bypass,
                    replica_groups=[list(range(NCORES))],
                    ins=[shard[:]],
                    outs=[full[:]],
                )

            def onehot(b, scaled=True):
                """Batched onehot for block b: [128slot, t_tot, 128dst]."""
                oh = ohp.tile([128, t_tot, 128], dt.bfloat16, tag="oh")
                a0, a1 = broadcast_tensor_aps(
                    iota_t[:], dl_t[:, b * t_tot : (b + 1) * t_tot, :]
                )
                nc.vector.tensor_tensor(out=oh[:], in0=a0, in1=a1, op=AOP.is_equal)
                if not scaled:
                    return oh
                ohs = ohsp.tile([128, t_tot, 128], dt.bfloat16, tag="ohs")
                c0, c1 = broadcast_tensor_aps(
                    oh[:], iv_t[:, b * t_tot : (b + 1) * t_tot, :]
                )
                nc.vector.tensor_tensor(out=ohs[:], in0=c0, in1=c1, op=AOP.mult)
                return ohs

            def scatter(yg, ohs):
                """t_tot onehot matmuls -> pagg [128feat, 128dst] (inv-scaled)."""
                pagg = ps.tile([128, 128], dt.float32, space="PSUM", tag="pagg")
                for cc in range(t_tot):
                    nc.tensor.matmul(
                        out=pagg[:],
                        lhsT=yg[:, cc, :],
                        rhs=ohs[:, cc, :],
                        start=(cc == 0),
                        stop=(cc == t_tot - 1),
                    )
                return pagg

            def transpose_to(dst_col0, src_nm, rows):
                pt = ps.tile([rows, 128], dt.bfloat16, space="PSUM", tag="pt")
                nc.tensor.transpose(out=pt[:], in_=src_nm, identity=ident_t[:])
                nc.vector.tensor_copy(out=dst_col0, in_=pt[:])

            # ================= Phase 1: xp + L1 =================
            # Transposes to xpT/h1T are hoisted below the loop: they stall PE
            # on an ACT round-trip per block, and L2 only needs them after the
            # T2 AllGather anyway.
            for b in range(NBLK):
                yg1 = stp.tile([128, t_tot, 128], dt.bfloat16, tag="yg1")
                nc.sync.dma_start(
                    yg1[:], stream1_in[:, b * t_tot : (b + 1) * t_tot, :]
                )
                ohs = onehot(b, scaled=False)  # stream rows pre-scaled by 1/deg
                pagg = scatter(yg1, ohs)
                xtb = xt_t[:, b * P : (b + 1) * P]
                pxp = ps.tile([128, O1], dt.float32, space="PSUM", tag="pm")
                nc.tensor.matmul(out=pxp[:], lhsT=xtb, rhs=wp_t[:], start=True, stop=False)
                nc.tensor.matmul(out=pxp[:], lhsT=ones_t[:], rhs=bp_t[:], start=False, stop=True)
                nc.scalar.activation(
                    out=t2sb[:, b, 64:128], in_=pxp[:], func=mybir.ActivationFunctionType.Relu
                )
                sx = sb.tile([128, 128], dt.bfloat16, tag="sx")
                nc.vector.tensor_copy(out=sx[:], in_=pagg[:])
                pm = ps.tile([128, O1], dt.float32, space="PSUM", tag="pm")
                nc.tensor.matmul(out=pm[:], lhsT=sx[:], rhs=w1c_t[:], start=True, stop=False)
                nc.tensor.matmul(out=pm[:], lhsT=xtb, rhs=wr1_t[:], start=False, stop=False)
                nc.tensor.matmul(out=pm[:], lhsT=ones_t[:], rhs=bl1_t[:], start=False, stop=True)
                nc.scalar.activation(
                    out=t2sb[:, b, 0:64], in_=pm[:], func=mybir.ActivationFunctionType.Relu
                )
                store_block(t2_shard, t2sb, b)

            for b in range(NBLK):
                transpose_to(h1T[:, b * P : (b + 1) * P], t2sb[:, b, 0:64], O1)
                transpose_to(xpT[:, b * P : (b + 1) * P], t2sb[:, b, 64:128], O1)

            if LAYERS == 1:
                for b in range(NBLK):
                    rows = min(P, SHARD - b * P)
                    nc.gpsimd.dma_start(
                        out=h3_out[b * P : b * P + rows, :], in_=t2sb[:rows, b, :]
                    )

            if LAYERS >= 2:
                allgather(t2_shard, t2_full)

                # ================= Phase 2: L2 =================
                for b in range(NBLK):
                    yg = yga if b % 2 == 0 else ygb
                    gather_block(yg, b, t2_full)
                    ohs = onehot(b)
                    pagg = scatter(yg, ohs)
                    nc.vector.tensor_copy(
                        out=s12T[:, b * P : (b + 1) * P], in_=pagg[:]
                    )
                    pm2 = ps.tile([128, O2], dt.float32, space="PSUM", tag="pm")
                    nc.tensor.matmul(
                        out=pm2[:], lhsT=s12T[:, b * P : (b + 1) * P], rhs=w2c_t[:],
                        start=True, stop=False,
                    )
                    nc.tensor.matmul(
                        out=pm2[:], lhsT=xpT[:, b * P : (b + 1) * P], rhs=wr2a_t[:],
                        start=False, stop=False,
                    )
                    nc.tensor.matmul(
                        out=pm2[:], lhsT=h1T[:, b * P : (b + 1) * P], rhs=wr2b_t[:],
                        start=False, stop=False,
                    )
                    nc.tensor.matmul(
                        out=pm2[:], lhsT=ones_t[:], rhs=bl2_t[:], start=False, stop=True
                    )
                    nc.scalar.activation(
                        out=t3sb[:, b, :], in_=pm2[:], func=mybir.ActivationFunctionType.Relu
                    )
                    transpose_to(h2T[:, b * P : (b + 1) * P], t3sb[:, b, :], O2)
                    store_block(t3_shard, t3sb, b)

            if LAYERS == 2:
                for b in range(NBLK):
                    rows = min(P, SHARD - b * P)
                    nc.gpsimd.dma_start(
                        out=h3_out[b * P : b * P + rows, :], in_=t3sb[:rows, b, :]
                    )

            if LAYERS >= 3:
                allgather(t3_shard, t3_full)

                # ================= Phase 3: L3 =================
                for b in range(NBLK):
                    yg = yga if b % 2 == 0 else ygb
                    gather_block(yg, b, t3_full)
                    ohs = onehot(b)
                    pagg = scatter(yg, ohs)
                    sh2 = sb.tile([128, 128], dt.bfloat16, tag="sx")
                    nc.vector.tensor_copy(out=sh2[:], in_=pagg[:])
                    pm3 = ps.tile([128, O3], dt.float32, space="PSUM", tag="pm")
                    nc.tensor.matmul(
                        out=pm3[:], lhsT=s12T[:, b * P : (b + 1) * P], rhs=w3c_t[:],
                        start=True, stop=False,
                    )
                    nc.tensor.matmul(
                        out=pm3[:], lhsT=sh2[:], rhs=wl3h2_t[:], start=False, stop=False
                    )
                    nc.tensor.matmul(
                        out=pm3[:], lhsT=xpT[:, b * P : (b + 1) * P], rhs=wr3a_t[:],
                        start=False, stop=False,
                    )
                    nc.tensor.matmul(
                        out=pm3[:], lhsT=h1T[:, b * P : (b + 1) * P], rhs=wr3b_t[:],
                        start=False, stop=False,
                    )
                    nc.tensor.matmul(
                        out=pm3[:], lhsT=h2T[:, b * P : (b + 1) * P], rhs=wr3c_t[:],
                        start=False, stop=False,
                    )
                    nc.tensor.matmul(
                        out=pm3[:], lhsT=ones_t[:], rhs=bl3_t[:], start=False, stop=True
                    )
                    h3t = sb.tile([128, O3], dt.float32, tag="h3t")
                    nc.scalar.activation(
                        out=h3t[:], in_=pm3[:], func=mybir.ActivationFunctionType.Relu
                    )
                    rows = min(P, SHARD - b * P)
                    nc.sync.dma_start(h3_out[b * P : b * P + rows, :], h3t[:rows, :])

    nc.compile()
    return nc


_NC_CACHE: dict = {}


def _make_in_maps(inputs, cores, t_lo, t_hi):
    t_tot = t_lo + t_hi
    TS = NBLK * t_tot * P
    x = np.asarray(inputs["x"], np.float32)

    iota = np.ascontiguousarray(
        np.broadcast_to(
            np.arange(128, dtype=np.float32), (128, t_tot, 128)
        ).astype(BF16)
    )
    ident = np.eye(128, dtype=np.float32)

    def wrow(v):
        return np.ascontiguousarray(np.asarray(v).reshape(1, -1))

    common = dict(
        iota3=iota,
        ident=_bf16(ident),
        Wp=_bf16(inputs["Wp"]),
        bp=_bf16(wrow(inputs["bp"])),
        # L1 stream rows [x_hi | x_lo] -> both multiply Wl1
        W1c=_bf16(np.vstack([inputs["Wl1"], inputs["Wl1"]])),
        bl1=_bf16(wrow(inputs["bl1"])),
        Wr1=_bf16(inputs["Wr1"]),
        # T2 col order [h1 | xp] -> [Wl2[64:], Wl2[:64]]
        W2c=_bf16(np.vstack([inputs["Wl2"][64:128], inputs["Wl2"][0:64]])),
        bl2=_bf16(wrow(inputs["bl2"])),
        Wr2a=_bf16(inputs["Wr2"][0:64]),
        Wr2b=_bf16(inputs["Wr2"][64:128]),
        # s12T rows [mean_h1; mean_xp] -> [Wl3[64:128], Wl3[0:64]]
        W3c=_bf16(np.vstack([inputs["Wl3"][64:128], inputs["Wl3"][0:64]])),
        Wl3h2=_bf16(inputs["Wl3"][128:256]),
        bl3=_bf16(wrow(inputs["bl3"])),
        Wr3a=_bf16(inputs["Wr3"][0:64]),
        Wr3b=_bf16(inputs["Wr3"][64:128]),
        Wr3c=_bf16(inputs["Wr3"][128:256]),
    )

    in_maps = []
    for c in range(NCORES):
        cc = cores[c]
        # L1 edge-major stream in slot layout, wrapped [128, NBLK*t_tot, 128].
        # Rows are pre-scaled by 1/deg(dst) (so L1 needs no inv multiply) and
        # split hi/lo around the SCALED value for near-f32 accuracy.
        xs = x[cc["srcs"]] * cc["sinv"][:, None]
        s_hi = xs.astype(BF16)
        s_lo = (xs - s_hi.astype(np.float32)).astype(BF16)
        stream = np.zeros((TS, FT), BF16)
        stream[cc["slots"], 0:64] = s_hi
        stream[cc["slots"], 64:128] = s_lo
        stream = np.ascontiguousarray(
            stream.reshape(NBLK * t_tot, P, FT).transpose(1, 0, 2)
        )
        xt = np.zeros((F0, SHARD_PAD), np.float32)
        xt[:, :SHARD] = x[c * SHARD : (c + 1) * SHARD].T
        in_maps.append(
            dict(
                common,
                stream1=stream,
                xt=_bf16(xt),
                idx=cc["idx"],
                dl=cc["dl"],
                iv=cc["iv"],
            )
        )
    return in_maps


def kernel(**inputs: np.ndarray) -> np.ndarray:
    edge_index = np.asarray(inputs["edge_index"])
    cores, t_lo, t_hi, nbh = _preprocess(edge_index)

    ck = (t_lo, t_hi, nbh.tobytes())
    if ck not in _NC_CACHE:
        _NC_CACHE[ck] = _build_nc(t_lo, t_hi, nbh)
    nc = _NC_CACHE[ck]

    in_maps = _make_in_maps(inputs, cores, t_lo, t_hi)
    res = run_bass_kernel_spmd(nc, in_maps, core_ids=list(range(NCORES)))
    out = np.concatenate([res.results[c]["h3"] for c in range(NCORES)], axis=0)
    return out.astype(np.float32)


# revision 46
# speedup vs baseline: 1.0235x; 1.0235x over previous
"""Trainium2 Bass kernel for 3-layer GraphSAGE-mean (DenseGCN variant).

Strategy (8 NeuronCores, graph/data parallel, v2):
  - Nodes sharded by range (6250/core); edges binned by (dst 128-block,
    src half) with uniform per-(block,half) chunk budgets t_lo/t_hi
    (int16 gather-index limit forces the src-half split).
  - L1 does NOT gather on device: the host bakes the edge-major stream
    [x_hi|x_lo][src] in exact slot layout; the kernel streams it with
    sequential HWDGE DMA (no Q7 descriptor generation).
  - The L1 stream rows are pre-scaled by 1/deg(dst) on the host (hi/lo
    bf16 split around the scaled value), so L1 uses a plain 0/1 onehot.
  - L2/L3 aggregate via gpsimd.dma_gather of 256B bf16 rows from the
    AllGathered tables T2=[h1|xp], T3=[h2]. Gather Q7 descriptor-gen
    time scales with the static num_idxs (~10ns/row) and is the kernel's
    critical path; pads use idx 0 and are killed by dstloc=-1.
  - Scatter = onehot matmuls accumulating [feat, dst] sums in PSUM.
    Onehots are generated BATCHED (one is_equal tensor_tensor per block
    over [128, t_tot, 128] with broadcast dstloc) and for L2/L3 scaled
    by 1/deg(dst) via a second batched mult, so the PSUM result is the
    mean directly; Wl/Wr/bias paths then accumulate into a single PSUM
    tile and the epilogue is just a ReLU.
  - Means of h1/xp computed at L2 (s12T) are reused at L3. Transposes
    to feature-major (xpT/h1T) are hoisted out of the L1 block loop to
    keep PE runs long (p-state ramp).
"""

import sys

sys.path.insert(0, "/opt/trn_rl_repo")

import os

import numpy as np
import ml_dtypes

import concourse.bass as bass
import concourse.bacc as bacc
import concourse.tile as tile
from concourse import mybir
from concourse.bass import broadcast_tensor_aps
from concourse.bass_utils import run_bass_kernel_spmd

BF16 = ml_dtypes.bfloat16

N = 50000
E = 800000
NCORES = 8
SHARD = N // NCORES          # 6250
P = 128
NBLK = (SHARD + P - 1) // P  # 49
SHARD_PAD = NBLK * P         # 6272
HALF = N // 2                # 25000

F0 = 64
O1, O2, O3 = 64, 128, 128
FT = 128                     # table feature width (bf16 -> 256B rows)
LAYERS = int(os.environ.get("KERNEL_LAYERS", "3"))

# Max chunks (x128 rows) per dma_gather call. 128 rows = 8 descriptors per
# SDMA engine. Must be small enough that every call keeps at least one
# valid (non-pad) slot on every core (per-call exact counts are baked on
# the host), and large enough to amortize the ~0.7us fixed Q7 cost.
GSPLIT = int(os.environ.get("KERNEL_GSPLIT", "5"))


def _call_ranges(t_lo: int, t_hi: int):
    """Chunk ranges [(c0, c1), ...] for one block's gather calls (lo then hi)."""
    t_tot = t_lo + t_hi
    out = []
    for c0, c1 in ((0, t_lo), (t_lo, t_tot)):
        step = (c1 - c0) if not GSPLIT else GSPLIT
        for s0 in range(c0, c1, step):
            out.append((s0, min(s0 + step, c1)))
    return out

dt = mybir.dt


def _bf16(x):
    return np.ascontiguousarray(np.asarray(x).astype(BF16))


def _wrap16(idx_flat: np.ndarray) -> np.ndarray:
    """[n] -> [128, n/16]: slot i at [i%16, i//16], replicated over 8 q7 cores."""
    n = idx_flat.shape[0]
    w = idx_flat.reshape(n // 16, 16).T
    return np.ascontiguousarray(np.tile(w, (8, 1)))


def _preprocess(edge_index: np.ndarray):
    """Bin edges -> per-core slot layout: gather idx, dstloc, slotinv, and
    the edge->slot permutation for baking the L1 stream."""
    src = edge_index[0].astype(np.int64)
    dst = edge_index[1].astype(np.int64)
    cnt = np.bincount(dst, minlength=N)
    inv = (1.0 / np.maximum(cnt, 1)).astype(np.float32)

    core = dst // SHARD
    blk = (dst % SHARD) // P
    half = (src >= HALF).astype(np.int64)
    lsrc = src - half * HALF
    key = (core * NBLK + blk) * 2 + half
    counts = np.bincount(key, minlength=NCORES * NBLK * 2)
    t_lo = int(np.ceil(counts[0::2].max() / P))
    t_hi = int(np.ceil(counts[1::2].max() / P))
    t_tot = t_lo + t_hi
    TS = NBLK * t_tot * P    # slots per core
    # per-(block, half) slot budget = max count over cores (NOT rounded to
    # 128): gather gen time scales with the static num_idxs, so trimming
    # the budget trims Q7 time directly. Baked into the program (SPMD).
    nbh = counts.reshape(NCORES, NBLK, 2).max(axis=0).astype(np.int64)

    order = np.argsort(key, kind="stable")
    skey = key[order]
    ssrc = src[order]
    slsrc = lsrc[order]
    sdst = dst[order]
    group_start = np.searchsorted(skey, np.arange(NCORES * NBLK * 2))
    pos = np.arange(E) - group_start[skey]

    scoreb = skey // 2
    shalf = skey % 2
    slot = (scoreb % NBLK) * (t_tot * P) + shalf * (t_lo * P) + pos
    score = scoreb // NBLK

    cores = []
    for c in range(NCORES):
        m = score == c
        sl = slot[m]
        # pad idx = 0 (valid row; contribution killed by dl=-1). Gather gen
        # time scales with the static num_idxs, so pads aren't worth skipping.
        sidx = np.zeros(TS, np.int16)
        local_src = slsrc[m]
        assert local_src.max(initial=0) < 32768
        sidx[sl] = local_src.astype(np.int16)
        sdl = np.full(TS, -1.0, np.float32)
        sdl[sl] = (sdst[m] % SHARD % P).astype(np.float32)
        sinv = np.zeros(TS, np.float32)
        sinv[sl] = inv[sdst[m]]

        # wrap idx per (block, half) region
        sidx3 = sidx.reshape(NBLK, t_tot * P)
        wr = np.empty((128, NBLK * t_tot * 8), np.int16)
        for b in range(NBLK):
            lo = _wrap16(sidx3[b, : t_lo * P])
            hi = _wrap16(sidx3[b, t_lo * P :])
            wr[:, b * t_tot * 8 : b * t_tot * 8 + t_lo * 8] = lo
            wr[:, b * t_tot * 8 + t_lo * 8 : (b + 1) * t_tot * 8] = hi

        # dl/iv [128, NBLK*t_tot, 1]: [p, b*t_tot+cc, 0] = val of slot cc*128+p
        dl = np.ascontiguousarray(
            sdl.reshape(NBLK * t_tot, P).T[:, :, None].astype(BF16)
        )
        iv = np.ascontiguousarray(
            sinv.reshape(NBLK * t_tot, P).T[:, :, None].astype(BF16)
        )
        # edge -> slot map for stream baking
        cores.append(
            dict(
                idx=wr,
                dl=dl,
                iv=iv,
                slots=sl,
                srcs=ssrc[m],
                sinv=inv[sdst[m]],
            )
        )

    return cores, t_lo, t_hi, nbh


def _build_nc(t_lo: int, t_hi: int, nbh: np.ndarray):
    t_tot = t_lo + t_hi
    nc = bacc.Bacc("TRN2", target_bir_lowering=False, debug=False)

    # ---- I/O ----
    stream1_in = nc.dram_tensor(
        "stream1", [128, NBLK * t_tot, FT], dt.bfloat16, kind="ExternalInput"
    )
    xt_in = nc.dram_tensor("xt", [F0, SHARD_PAD], dt.bfloat16, kind="ExternalInput")
    idx_in = nc.dram_tensor(
        "idx", [128, NBLK * t_tot * 8], dt.int16, kind="ExternalInput"
    )
    dl_in = nc.dram_tensor("dl", [128, NBLK * t_tot, 1], dt.bfloat16, kind="ExternalInput")
    iv_in = nc.dram_tensor("iv", [128, NBLK * t_tot, 1], dt.bfloat16, kind="ExternalInput")
    ranges = _call_ranges(t_lo, t_hi)
    iota_in = nc.dram_tensor("iota3", [128, t_tot, 128], dt.bfloat16, kind="ExternalInput")
    ident_in = nc.dram_tensor("ident", [128, 128], dt.bfloat16, kind="ExternalInput")

    wp_in = nc.dram_tensor("Wp", [F0, O1], dt.bfloat16, kind="ExternalInput")
    bp_in = nc.dram_tensor("bp", [1, O1], dt.bfloat16, kind="ExternalInput")
    w1c_in = nc.dram_tensor("W1c", [128, O1], dt.bfloat16, kind="ExternalInput")
    bl1_in = nc.dram_tensor("bl1", [1, O1], dt.bfloat16, kind="ExternalInput")
    wr1_in = nc.dram_tensor("Wr1", [F0, O1], dt.bfloat16, kind="ExternalInput")
    w2c_in = nc.dram_tensor("W2c", [128, O2], dt.bfloat16, kind="ExternalInput")
    bl2_in = nc.dram_tensor("bl2", [1, O2], dt.bfloat16, kind="ExternalInput")
    wr2a_in = nc.dram_tensor("Wr2a", [64, O2], dt.bfloat16, kind="ExternalInput")
    wr2b_in = nc.dram_tensor("Wr2b", [64, O2], dt.bfloat16, kind="ExternalInput")
    w3c_in = nc.dram_tensor("W3c", [128, O3], dt.bfloat16, kind="ExternalInput")
    wl3h2_in = nc.dram_tensor("Wl3h2", [128, O3], dt.bfloat16, kind="ExternalInput")
    bl3_in = nc.dram_tensor("bl3", [1, O3], dt.bfloat16, kind="ExternalInput")
    wr3a_in = nc.dram_tensor("Wr3a", [64, O3], dt.bfloat16, kind="ExternalInput")
    wr3b_in = nc.dram_tensor("Wr3b", [64, O3], dt.bfloat16, kind="ExternalInput")
    wr3c_in = nc.dram_tensor("Wr3c", [128, O3], dt.bfloat16, kind="ExternalInput")

    h3_out = nc.dram_tensor("h3", [SHARD, O3], dt.float32, kind="ExternalOutput")

    AOP = mybir.AluOpType

    with tile.TileContext(nc) as tc:
        with (
            tc.tile_pool(name="cons", bufs=1) as cons,
            tc.tile_pool(name="st", bufs=2) as stp,
            tc.tile_pool(name="ohp", bufs=2) as ohp,
            tc.tile_pool(name="ohsp", bufs=3) as ohsp,
            tc.tile_pool(name="sb", bufs=2) as sb,
            tc.tile_pool(name="psum", bufs=2, space="PSUM") as ps,
            tc.tile_pool(name="dram", bufs=1, space="DRAM") as dr,
        ):
            # ---- constants -> SBUF ----
            iota_t = cons.tile([128, t_tot, 128], dt.bfloat16)
            nc.sync.dma_start(iota_t[:], iota_in[:])
            ident_t = cons.tile([128, 128], dt.bfloat16)
            nc.sync.dma_start(ident_t[:], ident_in[:])
            idx_t = cons.tile([128, NBLK * t_tot * 8], dt.int16)
            nc.sync.dma_start(idx_t[:], idx_in[:])
            dl_t = cons.tile([128, NBLK * t_tot, 1], dt.bfloat16)
            nc.sync.dma_start(dl_t[:], dl_in[:])
            iv_t = cons.tile([128, NBLK * t_tot, 1], dt.bfloat16)
            nc.sync.dma_start(iv_t[:], iv_in[:])
            xt_t = cons.tile([F0, SHARD_PAD], dt.bfloat16)
            nc.sync.dma_start(xt_t[:], xt_in[:])
            ones_t = cons.tile([1, 128], dt.bfloat16)
            nc.gpsimd.memset(ones_t[:], 1.0)

            def load_w(name, src, shape):
                t = cons.tile(list(shape), dt.bfloat16, name=name)
                nc.sync.dma_start(t[:], src[:])
                return t

            wp_t = load_w("wp_t", wp_in, (F0, O1))
            bp_t = load_w("bp_t", bp_in, (1, O1))
            w1c_t = load_w("w1c_t", w1c_in, (128, O1))
            bl1_t = load_w("bl1_t", bl1_in, (1, O1))
            wr1_t = load_w("wr1_t", wr1_in, (F0, O1))
            w2c_t = load_w("w2c_t", w2c_in, (128, O2))
            bl2_t = load_w("bl2_t", bl2_in, (1, O2))
            wr2a_t = load_w("wr2a_t", wr2a_in, (64, O2))
            wr2b_t = load_w("wr2b_t", wr2b_in, (64, O2))
            w3c_t = load_w("w3c_t", w3c_in, (128, O3))
            wl3h2_t = load_w("wl3h2_t", wl3h2_in, (128, O3))
            bl3_t = load_w("bl3_t", bl3_in, (1, O3))
            wr3a_t = load_w("wr3a_t", wr3a_in, (64, O3))
            wr3b_t = load_w("wr3b_t", wr3b_in, (64, O3))
            wr3c_t = load_w("wr3c_t", wr3c_in, (128, O3))

            # ---- persistent feature-major activations (local shard) ----
            xpT = cons.tile([O1, SHARD_PAD], dt.bfloat16)
            h1T = cons.tile([O1, SHARD_PAD], dt.bfloat16)
            h2T = cons.tile([O2, SHARD_PAD], dt.bfloat16)
            s12T = cons.tile([128, SHARD_PAD], dt.bfloat16)  # [mean_h1; mean_xp]
            t2sb = cons.tile([128, NBLK, 128], dt.bfloat16)  # staging [h1|xp]
            t3sb = cons.tile([128, NBLK, 128], dt.bfloat16)  # staging [h2]

            # gather double buffers (memset once: pad slots are never written)
            yga = cons.tile([128, t_tot, 128], dt.bfloat16)
            ygb = cons.tile([128, t_tot, 128], dt.bfloat16)
            nc.gpsimd.memset(yga[:], 0.0)
            nc.gpsimd.memset(ygb[:], 0.0)

            # ---- DRAM tables ----
            t2_shard = dr.tile([SHARD, FT], dt.bfloat16)
            t2_full = dr.tile([N, FT], dt.bfloat16, addr_space="Shared")
            t3_shard = dr.tile([SHARD, FT], dt.bfloat16)
            t3_full = dr.tile([N, FT], dt.bfloat16, addr_space="Shared")

            def gather_block(yg, b, table):
                # NOTE: gather gen time does NOT drop with smaller num_idxs
                # (measured: trimming to per-block exact counts was ~40us
                # slower) -- keep full-chunk calls.
                ibase = b * t_tot * 8
                for s0, s1 in ranges:
                    tbl = table[0:HALF, :] if s0 < t_lo else table[HALF:N, :]
                    nc.gpsimd.dma_gather(
                        yg[:, s0:s1, :],
                        tbl,
                        idx_t[:, ibase + s0 * 8 : ibase + s1 * 8],
                        (s1 - s0) * P,
                        (s1 - s0) * P,
                        FT,
                    )

            def store_block(shard, staging, b):
                rows = min(P, SHARD - b * P)
                nc.sync.dma_start(
                    shard[b * P : b * P + rows, :], staging[:rows, b, :]
                )

            def allgather(shard, full):
                nc.gpsimd.collective_compute(
                    "AllGather",
                    mybir.AluOpType.bypass,
                    replica_groups=[list(range(NCORES))],
                    ins=[shard[:]],
                    outs=[full[:]],
                )

            def onehot(b, scaled=True):
                """Batched onehot for block b: [128slot, t_tot, 128dst]."""
                oh = ohp.tile([128, t_tot, 128], dt.bfloat16, tag="oh")
                a0, a1 = broadcast_tensor_aps(
                    iota_t[:], dl_t[:, b * t_tot : (b + 1) * t_tot, :]
                )
                nc.vector.tensor_tensor(out=oh[:], in0=a0, in1=a1, op=AOP.is_equal)
                if not scaled:
                    return oh
                ohs = ohsp.tile([128, t_tot, 128], dt.bfloat16, tag="ohs")
                c0, c1 = broadcast_tensor_aps(
                    oh[:], iv_t[:, b * t_tot : (b + 1) * t_tot, :]
                )
                nc.vector.tensor_tensor(out=ohs[:], in0=c0, in1=c1, op=AOP.mult)
                return ohs

            def scatter(yg, ohs):
                """t_tot onehot matmuls -> pagg [128feat, 128dst] (inv-scaled)."""
                pagg = ps.tile([128, 128], dt.float32, space="PSUM", tag="pagg")
                for cc in range(t_tot):
                    nc.tensor.matmul(
                        out=pagg[:],
                        lhsT=yg[:, cc, :],
                        rhs=ohs[:, cc, :],
                        start=(cc == 0),
                        stop=(cc == t_tot - 1),
                    )
                return pagg

            def transpose_to(dst_col0, src_nm, rows):
                pt = ps.tile([rows, 128], dt.bfloat16, space="PSUM", tag="pt")
                nc.tensor.transpose(out=pt[:], in_=src_nm, identity=ident_t[:])
                nc.vector.tensor_copy(out=dst_col0, in_=pt[:])

            # ================= Phase 1: xp + L1 =================
            # Transposes to xpT/h1T are hoisted below the loop: they stall PE
            # on an ACT round-trip per block, and L2 only needs them after the
            # T2 AllGather anyway.
            for b in range(NBLK):
                yg1 = stp.tile([128, t_tot, 128], dt.bfloat16, tag="yg1")
                nc.sync.dma_start(
                    yg1[:], stream1_in[:, b * t_tot : (b + 1) * t_tot, :]
                )
                ohs = onehot(b, scaled=False)  # stream rows pre-scaled by 1/deg
                pagg = scatter(yg1, ohs)
                xtb = xt_t[:, b * P : (b + 1) * P]
                pxp = ps.tile([128, O1], dt.float32, space="PSUM", tag="pm")
                nc.tensor.matmul(out=pxp[:], lhsT=xtb, rhs=wp_t[:], start=True, stop=False)
                nc.tensor.matmul(out=pxp[:], lhsT=ones_t[:], rhs=bp_t[:], start=False, stop=True)
                nc.scalar.activation(
                    out=t2sb[:, b, 64:128], in_=pxp[:], func=mybir.ActivationFunctionType.Relu
                )
                sx = sb.tile([128, 128], dt.bfloat16, tag="sx")
                nc.vector.tensor_copy(out=sx[:], in_=pagg[:])
                pm = ps.tile([128, O1], dt.float32, space="PSUM", tag="pm")
                nc.tensor.matmul(out=pm[:], lhsT=sx[:], rhs=w1c_t[:], start=True, stop=False)
                nc.tensor.matmul(out=pm[:], lhsT=xtb, rhs=wr1_t[:], start=False, stop=False)
                nc.tensor.matmul(out=pm[:], lhsT=ones_t[:], rhs=bl1_t[:], start=False, stop=True)
                nc.scalar.activation(
                    out=t2sb[:, b, 0:64], in_=pm[:], func=mybir.ActivationFunctionType.Relu
                )
                store_block(t2_shard, t2sb, b)

            for b in range(NBLK):
                transpose_to(h1T[:, b * P : (b + 1) * P], t2sb[:, b, 0:64], O1)
                transpose_to(xpT[:, b * P : (b + 1) * P], t2sb[:, b, 64:128], O1)

            if LAYERS == 1:
                for b in range(NBLK):
                    rows = min(P, SHARD - b * P)
                    nc.gpsimd.dma_start(
                        out=h3_out[b * P : b * P + rows, :], in_=t2sb[:rows, b, :]
                    )

            if LAYERS >= 2:
                allgather(t2_shard, t2_full)

                # ================= Phase 2: L2 =================
                for b in range(NBLK):
                    yg = yga if b % 2 == 0 else ygb
                    gather_block(yg, b, t2_full)
                    ohs = onehot(b)
                    pagg = scatter(yg, ohs)
                    nc.vector.tensor_copy(
                        out=s12T[:, b * P : (b + 1) * P], in_=pagg[:]
                    )
                    pm2 = ps.tile([128, O2], dt.float32, space="PSUM", tag="pm")
                    nc.tensor.matmul(
                        out=pm2[:], lhsT=s12T[:, b * P : (b + 1) * P], rhs=w2c_t[:],
                        start=True, stop=False,
                    )
                    nc.tensor.matmul(
                        out=pm2[:], lhsT=xpT[:, b * P : (b + 1) * P], rhs=wr2a_t[:],
                        start=False, stop=False,
                    )
                    nc.tensor.matmul(
                        out=pm2[:], lhsT=h1T[:, b * P : (b + 1) * P], rhs=wr2b_t[:],
                        start=False, stop=False,
                    )
                    nc.tensor.matmul(
                        out=pm2[:], lhsT=ones_t[:], rhs=bl2_t[:], start=False, stop=True
                    )
                    nc.scalar.activation(
                        out=t3sb[:, b, :], in_=pm2[:], func=mybir.ActivationFunctionType.Relu
                    )
                    transpose_to(h2T[:, b * P : (b + 1) * P], t3sb[:, b, :], O2)
                    store_block(t3_shard, t3sb, b)

            if LAYERS == 2:
                for b in range(NBLK):
                    rows = min(P, SHARD - b * P)
                    nc.gpsimd.dma_start(
                        out=h3_out[b * P : b * P + rows, :], in_=t3sb[:rows, b, :]
                    )

            if LAYERS >= 3:
                allgather(t3_shard, t3_full)

                # ================= Phase 3: L3 =================
                for b in range(NBLK):
                    yg = yga if b % 2 == 0 else ygb
                    gather_block(yg, b, t3_full)
                    ohs = onehot(b)
                    pagg = scatter(yg, ohs)
                    sh2 = sb.tile([128, 128], dt.bfloat16, tag="sx")
                    nc.vector.tensor_copy(out=sh2[:], in_=pagg[:])
                    pm3 = ps.tile([128, O3], dt.float32, space="PSUM", tag="pm")
                    nc.tensor.matmul(
                        out=pm3[:], lhsT=s12T[:, b * P : (b + 1) * P], rhs=w3c_t[:],
                        start=True, stop=False,
                    )
                    nc.tensor.matmul(
                        out=pm3[:], lhsT=sh2[:], rhs=wl3h2_t[:], start=False, stop=False
                    )
                    nc.tensor.matmul(
                        out=pm3[:], lhsT=xpT[:, b * P : (b + 1) * P], rhs=wr3a_t[:],
                        start=False, stop=False,
                    )
                    nc.tensor.matmul(
                        out=pm3[:], lhsT=h1T[:, b * P : (b + 1) * P], rhs=wr3b_t[:],
                        start=False, stop=False,
                    )
                    nc.tensor.matmul(
                        out=pm3[:], lhsT=h2T[:, b * P : (b + 1) * P], rhs=wr3c_t[:],
                        start=False, stop=False,
                    )
                    nc.tensor.matmul(
                        out=pm3[:], lhsT=ones_t[:], rhs=bl3_t[:], start=False, stop=True
                    )
                    h3t = sb.tile([128, O3], dt.float32, tag="h3t")
                    nc.scalar.activation(
                        out=h3t[:], in_=pm3[:], func=mybir.ActivationFunctionType.Relu
                    )
                    rows = min(P, SHARD - b * P)
                    nc.sync.dma_start(h3_out[b * P : b * P + rows, :], h3t[:rows, :])

    nc.compile()
    return nc


_NC_CACHE: dict = {}


def _make_in_maps(inputs, cores, t_lo, t_hi):
    t_tot = t_lo + t_hi
    TS = NBLK * t_tot * P
    x = np.asarray(inputs["x"], np.float32)

    iota = np.ascontiguousarray(
        np.broadcast_to(
            np.arange(128, dtype=np.float32), (128, t_tot, 128)
        ).astype(BF16)
    )
    ident = np.eye(128, dtype=np.float32)

    def wrow(v):
        return np.ascontiguousarray(np.asarray(v).reshape(1, -1))

    common = dict(
        iota3=iota,
        ident=_bf16(ident),
        Wp=_bf16(inputs["Wp"]),
        bp=_bf16(wrow(inputs["bp"])),
        # L1 stream rows [x_hi | x_lo] -> both multiply Wl1
        W1c=_bf16(np.vstack([inputs["Wl1"], inputs["Wl1"]])),
        bl1=_bf16(wrow(inputs["bl1"])),
        Wr1=_bf16(inputs["Wr1"]),
        # T2 col order [h1 | xp] -> [Wl2[64:], Wl2[:64]]
        W2c=_bf16(np.vstack([inputs["Wl2"][64:128], inputs["Wl2"][0:64]])),
        bl2=_bf16(wrow(inputs["bl2"])),
        Wr2a=_bf16(inputs["Wr2"][0:64]),
        Wr2b=_bf16(inputs["Wr2"][64:128]),
        # s12T rows [mean_h1; mean_xp] -> [Wl3[64:128], Wl3[0:64]]
        W3c=_bf16(np.vstack([inputs["Wl3"][64:128], inputs["Wl3"][0:64]])),
        Wl3h2=_bf16(inputs["Wl3"][128:256]),
        bl3=_bf16(wrow(inputs["bl3"])),
        Wr3a=_bf16(inputs["Wr3"][0:64]),
        Wr3b=_bf16(inputs["Wr3"][64:128]),
        Wr3c=_bf16(inputs["Wr3"][128:256]),
    )

    in_maps = []
    for c in range(NCORES):
        cc = cores[c]
        # L1 edge-major stream in slot layout, wrapped [128, NBLK*t_tot, 128].
        # Rows are pre-scaled by 1/deg(dst) (so L1 needs no inv multiply) and
        # split hi/lo around the SCALED value for near-f32 accuracy.
        xs = x[cc["srcs"]] * cc["sinv"][:, None]
        s_hi = xs.astype(BF16)
        s_lo = (xs - s_hi.astype(np.float32)).astype(BF16)
        stream = np.zeros((TS, FT), BF16)
        stream[cc["slots"], 0:64] = s_hi
        stream[cc["slots"], 64:128] = s_lo
        stream = np.ascontiguousarray(
            stream.reshape(NBLK * t_tot, P, FT).transpose(1, 0, 2)
        )
        xt = np.zeros((F0, SHARD_PAD), np.float32)
        xt[:, :SHARD] = x[c * SHARD : (c + 1) * SHARD].T
        in_maps.append(
            dict(
                common,
                stream1=stream,
                xt=_bf16(xt),
                idx=cc["idx"],
                dl=cc["dl"],
                iv=cc["iv"],
            )
        )
    return in_maps


def kernel(**inputs: np.ndarray) -> np.ndarray:
    edge_index = np.asarray(inputs["edge_index"])
    cores, t_lo, t_hi, nbh = _preprocess(edge_index)

    ck = (t_lo, t_hi, nbh.tobytes())
    if ck not in _NC_CACHE:
        _NC_CACHE[ck] = _build_nc(t_lo, t_hi, nbh)
    nc = _NC_CACHE[ck]

    in_maps = _make_in_maps(inputs, cores, t_lo, t_hi)
    res = run_bass_kernel_spmd(nc, in_maps, core_ids=list(range(NCORES)))
    out = np.concatenate([res.results[c]["h3"] for c in range(NCORES)], axis=0)
    return out.astype(np.float32)
